# revision 20
# baseline (speedup 1.0000x reference)
"""Trainium2 Bass kernel for nn_ActivityAugmentation.

Reference pipeline (per sample b, time t, channel c):
  1. jitter:   xj = x + noise * 0.01
  2. scale:    * (0.9 + scale_u * 0.2)            [folded into warp weights]
  3. timewarp: y[t] = xj[i0[t]] * w0[t] + xj[i0[t]+1] * w1[t]
  4. rotation of channels 0,1 by per-sample angle  [commutes with 3]
  5. channel dropout mask

Sharding: pure data-parallel over batch, 64 samples per NeuronCore (8 cores).

Two device strategies:

v1 (baseline): W-stationary warp — slab [s%128, s//128, b, c], banded W as
  128x128 lhsT blocks, ~35 matmuls x 512-free per group, ACT psum evict.

v2: x-stationary warp. For each bc-chunk (2 samples x 64 ch = 128 psum
  partitions) the full T=2048 output lives in one 4-bank psum tile, produced
  by ~32 matmuls whose moving dim is a target interval: per-interval rhs
  weights [128, len] contract the jittered slab column (lhsT [128src, 128bc]).
  Total PE streaming = 2048 cycles/chunk (the floor; v1 pays 35*512/4 = 4480).
  Oscillation of i0 across block boundaries is handled by a 64-offset B-slab
  (built with 2 partition-shifted SBUF->SBUF DMAs) so every target is produced
  by exactly one matmul. Jitter is a DVE tensor_add of host-prescaled noise
  (bf16 2x mode; scalar_tensor_tensor has no DVE fast mode, and the gpsimd
  accumulate-DMA wedges the device for runs over ~1KB/partition); rotation
  stays on DVE in bf16; psum evict is split ACT/DVE at KERNEL_ECUT; stores
  issue from the idle gpsimd queue; all inputs bf16 with dropped channels
  pre-zeroed on host.
"""

import os
import numpy as np

import concourse.bacc as bacc
import concourse.mybir as mybir
from concourse.tile import TileContext
from concourse.bass_utils import run_bass_kernel_spmd

B, T, C = 512, 2048, 64
JITTER_STD = 0.01
SCALE_LO, SCALE_HI = 0.9, 1.1
TW_SIGMA = 0.2

N_CORES = 8
BS = B // N_CORES  # 64 batch samples per core
GB = 8             # batch samples per group (free dim = GB*C = 512)
NG = BS // GB      # 8 groups
P = 128
NTB = T // P       # 16 source blocks
F = GB * C         # 512
NCH = GB // 2      # bc-chunks per group in v2

F32 = mybir.dt.float32
F32R = mybir.dt.float32r
BF16 = mybir.dt.bfloat16


def _warp_params(warp_noise):
    """Replicate the reference's fp32 warp math on host (cheap, O(T))."""
    wn = np.asarray(warp_noise, dtype=np.float32)
    warp = np.cumsum(wn * np.float32(TW_SIGMA / T), dtype=np.float32)
    warp = (warp - warp[0]).astype(np.float32)
    warp = (warp / (warp[-1] + np.float32(1e-8))).astype(np.float32)
    t_orig = np.linspace(0.0, 1.0, T, dtype=np.float32)
    t_warped = np.clip(t_orig + warp * np.float32(0.2), np.float32(0.0), np.float32(1.0)).astype(np.float32)
    pos = (t_warped * np.float32(T - 1)).astype(np.float32)
    i0 = np.clip(np.floor(pos).astype(np.int32), 0, T - 2)
    frac = (pos - i0.astype(np.float32)).astype(np.float32)
    return i0, frac


# ---------------------------------------------------------------------------
# v1: W-stationary banded matmul (baseline)
# ---------------------------------------------------------------------------

def _build_w_blocks(i0, frac, scale):
    w0 = (scale * (np.float32(1.0) - frac)).astype(np.float32)
    w1 = (scale * frac).astype(np.float32)
    blocks = []
    sched = []
    for tb in range(NTB):
        tl = np.arange(tb * P, (tb + 1) * P)
        per_sb = {}
        for idx, wgt in ((i0[tl], w0[tl]), (i0[tl] + 1, w1[tl])):
            nz = wgt != 0.0
            for sb in np.unique(idx[nz] // P):
                m = nz & (idx // P == sb)
                blk = per_sb.setdefault(int(sb), np.zeros((P, P), np.float32))
                np.add.at(blk, (idx[m] - sb * P, np.arange(P)[m]), wgt[m])
        entry = []
        for sb in sorted(per_sb):
            entry.append((sb, len(blocks)))
            blocks.append(per_sb[sb])
        sched.append(entry)
    wmat = np.stack(blocks).astype(np.float32)
    return wmat, sched


def _build_nc_v1(nblk, sched, dropped, rot_needed, iters=1):
    dma_only = bool(int(os.environ.get("KERNEL_DMA_ONLY", "0")))
    skip = set(os.environ.get("KERNEL_SKIP", "").split(","))
    stt2 = int(os.environ.get("KERNEL_STT2", "2"))
    nc = bacc.Bacc(trn_type="TRN2")
    xin = nc.declare_dram_parameter("x", [BS, T, C], F32R, isOutput=False)
    nin = nc.declare_dram_parameter("n", [BS, T, C], BF16, isOutput=False)
    win = nc.declare_dram_parameter("w", [nblk, P, P], F32R, isOutput=False)
    rin = nc.declare_dram_parameter("rot", [2 * NG, P, P], F32, isOutput=False)
    out = nc.declare_dram_parameter("out", [NG, NTB, P, GB * C], F32, isOutput=True)

    with TileContext(nc) as tc:
        with (
            tc.tile_pool(name="consts", bufs=1) as cpool,
            tc.tile_pool(name="xs", bufs=int(os.environ.get("KERNEL_XBUFS", "2"))) as xpool,
            tc.tile_pool(name="ns", bufs=int(os.environ.get("KERNEL_NBUFS", "2"))) as npool,
            tc.tile_pool(name="tmp", bufs=2) as tpool,
            tc.tile_pool(name="ot", bufs=int(os.environ.get("KERNEL_OBUFS", "4"))) as opool,
            tc.tile_pool(
                name="psum", bufs=int(os.environ.get("KERNEL_PBUFS", "8")), space="PSUM"
            ) as ppool,
        ):
            wt = cpool.tile([P, nblk, P], F32R)
            nc.sync.dma_start(out=wt[:], in_=win.rearrange("k s t -> s k t"))
            rt = cpool.tile([P, 2, NG, P], F32)
            nc.sync.dma_start(
                out=rt[:].rearrange("p a g q -> p (a g) q"),
                in_=rin.rearrange("k p q -> p k q"),
            )

            for g in range(NG * iters):
                g = g % NG
                xs = xpool.tile([P, NTB, GB, C], F32R)
                ns = npool.tile([P, NTB, GB, C], BF16)
                for b in range(GB):
                    nc.sync.dma_start(
                        out=xs[:, :, b, :],
                        in_=xin[g * GB + b].rearrange("(sb p) c -> p sb c", p=P),
                    )
                    nc.sync.dma_start(
                        out=ns[:, :, b, :],
                        in_=nin[g * GB + b].rearrange("(sb p) c -> p sb c", p=P),
                    )
                if dma_only:
                    for tb in range(NTB):
                        nc.scalar.dma_start(
                            out=out[g, tb],
                            in_=xs[:, tb, :, :].bitcast(F32),
                        )
                    continue
                if "j" not in skip:
                    csz = NTB // stt2
                    for ci in range(stt2):
                        sl = slice(ci * csz, (ci + 1) * csz)
                        nc.vector.scalar_tensor_tensor(
                            out=xs[:, sl],
                            in0=ns[:, sl],
                            scalar=JITTER_STD,
                            in1=xs[:, sl],
                            op0=mybir.AluOpType.mult,
                            op1=mybir.AluOpType.add,
                        )
                if rot_needed and "r" not in skip:
                    ca = rt[:, 0, g, :].rearrange("p (q b) -> p q b", q=NTB)
                    sa = rt[:, 1, g, :].rearrange("p (q b) -> p q b", q=NTB)
                    u0 = xs[:, :, :, 0]
                    u1 = xs[:, :, :, 1]
                    tt = [
                        tpool.tile([P, NTB, GB], F32, tag=f"t{i}", name=f"t{i}_{g}")
                        for i in range(4)
                    ]
                    nc.vector.tensor_mul(out=tt[0][:], in0=u0, in1=ca)
                    nc.vector.tensor_mul(out=tt[1][:], in0=u1, in1=sa)
                    nc.vector.tensor_mul(out=tt[2][:], in0=u0, in1=sa)
                    nc.vector.tensor_mul(out=tt[3][:], in0=u1, in1=ca)
                    nc.vector.tensor_sub(out=u0, in0=tt[0][:], in1=tt[1][:])
                    nc.vector.tensor_add(out=u1, in0=tt[2][:], in1=tt[3][:])
                for c in (dropped if "r" not in skip else []):
                    nc.vector.tensor_scalar_mul(
                        xs[:, :, :, c], xs[:, :, :, c], 0.0
                    )
                if "m" in skip:
                    for tb in range(NTB):
                        nc.scalar.dma_start(
                            out=out[g, tb],
                            in_=xs[:, tb, :, :].bitcast(F32),
                        )
                    continue
                for tbq in range(NTB // 4):
                    ot = opool.tile([P, 4, F], F32, tag="ot4", name=f"ot4_{g}_{tbq}")
                    for k in range(4):
                        tb = tbq * 4 + k
                        ps = ppool.tile([P, F], F32, tag="ps", name=f"ps_{g}_{tb}")
                        n_mm = len(sched[tb])
                        for j, (sb, bi) in enumerate(sched[tb]):
                            nc.tensor.matmul(
                                ps[:],
                                wt[:, bi, :],
                                xs[:, sb, :, :],
                                start=(j == 0),
                                stop=(j == n_mm - 1),
                            )
                        nc.scalar.copy(out=ot[:, k, :], in_=ps[:])
                    nc.scalar.dma_start(
                        out=out[g, tbq * 4:(tbq + 1) * 4].rearrange("q p f -> p q f"),
                        in_=ot[:],
                    )
    nc.compile()
    return nc


# ---------------------------------------------------------------------------
# v2: x-stationary warp, accum-DMA jitter, split ACT/DVE evict
# ---------------------------------------------------------------------------

def _build_v2_schedule(i0, frac, scale):
    """Shared matmul schedule. Every target is produced by exactly one matmul
    (zones=b mode: oscillation zones read the 64-offset B-slab) or by at most
    two accumulating matmuls (zones=pair mode: zone covered by cols k and k+1
    of the aligned A-slab with start=True/False). Intervals are split at psum
    bank (512) boundaries; rhs weight offsets are 16-element aligned. Raises
    ValueError if the warp structure doesn't fit (caller falls back to v1).

    instr = (slab, col, woff, lo, hi, start) with slab 0=A, 1=B.
    """
    zmode = os.environ.get("KERNEL_V2_ZONES", "b")  # b | pair
    b0 = i0 // P
    b1 = (i0 + 1) // P
    if np.abs(np.diff(b0)).max() > 1:
        raise ValueError("b0 jumps by >1")
    nb = int(b0.max()) + 1
    zones = []
    for k in range(nb - 1):
        sel1 = np.nonzero(b0 == k + 1)[0]
        selk = np.nonzero(b0 == k)[0]
        strad = np.nonzero((b0 == k) & (b1 == k + 1))[0]
        if len(sel1) == 0 or len(selk) == 0:
            continue
        t_first, t_last = int(sel1.min()), int(selk.max())
        if t_first > t_last and len(strad) == 0:
            continue
        lo = min(t_first, int(strad.min()) if len(strad) else t_first)
        hi = max(t_last, int(strad.max()) if len(strad) else t_last)
        zones.append((k, lo, hi))
    for a, b in zip(zones, zones[1:]):
        if b[1] <= a[2]:
            raise ValueError("overlapping zones")
    if zmode == "b":
        for k, lo, hi in zones:
            ts = np.arange(lo, hi + 1)
            smin = int(i0[ts].min())
            smax = int(i0[ts].max()) + 1
            if smin < 128 * k + 64 or smax >= 128 * (k + 1) + 64:
                raise ValueError("zone taps escape B-window")
    zone_cols = [k for k, _, _ in zones] if zmode == "b" else []
    w0 = (scale * (np.float32(1.0) - frac)).astype(np.float32)
    w1 = (scale * frac).astype(np.float32)

    instrs = []
    wcols = []
    off = 0

    def emit_piece(slab, col, base, a, b, start, stop, masked):
        nonlocal off
        blk = np.zeros((P, b - a), np.float32)
        ts = np.arange(a, b)
        for idx, wgt in ((i0[ts], w0[ts]), (i0[ts] + 1, w1[ts])):
            m = (idx >= base) & (idx < base + P) if masked else slice(None)
            np.add.at(blk, (idx[m] - base, (ts - a)[m]), wgt[m])
        padded = -(-(b - a) // 16) * 16  # 16-elem (32B bf16) aligned offsets
        if padded != b - a:
            blk = np.concatenate([blk, np.zeros((P, padded - (b - a)), np.float32)], axis=1)
        instrs.append((slab, col, off, a, b, start, stop))
        wcols.append(blk)
        off += padded

    def pieces(lo, hi):
        cuts = [lo]
        c = (lo // 512 + 1) * 512
        while c < hi:
            cuts.append(c)
            c += 512
        cuts.append(hi)
        return list(zip(cuts, cuts[1:]))

    t = 0
    zi = 0
    while t < T:
        if zi < len(zones) and t == zones[zi][1]:
            k, lo, hi = zones[zi]
            for a, b in pieces(lo, hi + 1):
                if zmode == "b":
                    emit_piece(1, k, 128 * k + 64, a, b, True, True, False)
                else:
                    emit_piece(0, k, 128 * k, a, b, True, False, True)
                    emit_piece(0, k + 1, 128 * (k + 1), a, b, False, True, True)
            t = hi + 1
            zi += 1
            continue
        k = int(b0[t])
        e = t
        nxt = zones[zi][1] if zi < len(zones) else T
        while e < T and e < nxt and b0[e] == k:
            e += 1
        for a, b in pieces(t, e):
            emit_piece(0, k, 128 * k, a, b, True, True, False)
        t = e
    wmat = np.concatenate(wcols, axis=1)
    return wmat, instrs, zone_cols


def _build_nc_v2(instrs, zone_cols, lw, rot_needed, drop01, iters=1):
    # jitter mode: add = DVE tensor_add of host-prescaled noise (bf16 2x mode);
    # dma = gpsimd accumulate-DMA (wedges the device for descriptors over
    # ~1KB/partition - the CCE accumulate path is size-limited; do not use)
    jmode = os.environ.get("KERNEL_V2_JITTER", "add")
    ecut = int(os.environ.get("KERNEL_ECUT", "1856"))
    dma_only = bool(int(os.environ.get("KERNEL_DMA_ONLY", "0")))
    skip = set(os.environ.get("KERNEL_SKIP", "").split(","))
    outdt = os.environ.get("KERNEL_V2_OUT", "f32")
    ODT = F32 if outdt == "f32" else BF16
    nzone = len(zone_cols)
    klo = min(zone_cols) if zone_cols else 0
    khi = max(zone_cols) if zone_cols else -1
    ncol = khi - klo + 1
    nc = bacc.Bacc(trn_type="TRN2")
    xin = nc.declare_dram_parameter("x", [BS, T, C], BF16, isOutput=False)
    nin = nc.declare_dram_parameter("n", [BS, T, C], BF16, isOutput=False)
    win = nc.declare_dram_parameter("w", [P, lw], BF16, isOutput=False)
    rin = nc.declare_dram_parameter("rot", [2 * NG, P, P], BF16, isOutput=False)
    out = nc.declare_dram_parameter("out", [NG, NCH, P, T], ODT, isOutput=True)

    with TileContext(nc) as tc:
        with (
            tc.tile_pool(name="consts", bufs=1) as cpool,
            tc.tile_pool(name="xs", bufs=int(os.environ.get("KERNEL_XBUFS", "2"))) as xpool,
            tc.tile_pool(name="ns", bufs=int(os.environ.get("KERNEL_NBUFS", "2"))) as npool,
            tc.tile_pool(name="xb", bufs=2) as bpool,
            tc.tile_pool(name="tmp", bufs=2) as tpool,
            tc.tile_pool(name="ot", bufs=int(os.environ.get("KERNEL_OBUFS", "3"))) as opool,
            tc.tile_pool(name="psum", bufs=2, space="PSUM") as ppool,
        ):
            wt = cpool.tile([P, lw], BF16)
            nc.sync.dma_start(out=wt[:], in_=win[:, :])
            rt = cpool.tile([P, 2, NG, P], BF16)
            nc.sync.dma_start(
                out=rt[:].rearrange("p a g q -> p (a g) q"),
                in_=rin.rearrange("k p q -> p k q"),
            )
            for g in range(NG * iters):
                g = g % NG
                xs = xpool.tile([P, NTB, GB, C], BF16)
                ns = npool.tile([P, NTB, GB, C], BF16)
                for b in range(GB):
                    nc.sync.dma_start(
                        out=xs[:, :, b, :],
                        in_=xin[g * GB + b].rearrange("(sb p) c -> p sb c", p=P),
                    )
                    if jmode != "none":
                        nc.sync.dma_start(
                            out=ns[:, :, b, :],
                            in_=nin[g * GB + b].rearrange("(sb p) c -> p sb c", p=P),
                        )
                if dma_only:
                    for ch in range(NCH):
                        nc.gpsimd.dma_start(
                            out=out[g, ch],
                            in_=xs[:, 4 * ch:4 * ch + 4].rearrange(
                                "p a b c -> p (a b c)"
                            ).bitcast(F32),
                        )
                    continue
                if jmode == "add" and "j" not in skip:
                    nc.vector.tensor_add(out=xs[:], in0=xs[:], in1=ns[:])
                elif jmode == "dma" and "j" not in skip:
                    nc.gpsimd.dma_start(
                        out=xs[:].rearrange("p a b c -> p (a b c)"),
                        in_=ns[:].rearrange("p a b c -> p (a b c)"),
                        accum_op=mybir.AluOpType.add,
                    )
                if rot_needed and "r" not in skip:
                    ca = rt[:, 0, g, :].rearrange("p (q b) -> p q b", q=NTB)
                    sa = rt[:, 1, g, :].rearrange("p (q b) -> p q b", q=NTB)
                    u0 = xs[:, :, :, 0]
                    u1 = xs[:, :, :, 1]
                    tt = [
                        tpool.tile([P, NTB, GB], BF16, tag=f"t{i}", name=f"t{i}_{g}")
                        for i in range(4)
                    ]
                    nc.vector.tensor_mul(out=tt[0][:], in0=u0, in1=ca)
                    nc.vector.tensor_mul(out=tt[1][:], in0=u1, in1=sa)
                    nc.vector.tensor_mul(out=tt[2][:], in0=u0, in1=sa)
                    nc.vector.tensor_mul(out=tt[3][:], in0=u1, in1=ca)
                    nc.vector.tensor_sub(out=u0, in0=tt[0][:], in1=tt[1][:])
                    nc.vector.tensor_add(out=u1, in0=tt[2][:], in1=tt[3][:])
                    for c in drop01:
                        nc.vector.tensor_scalar_mul(xs[:, :, :, c], xs[:, :, :, c], 0.0)
                if nzone and "m" not in skip:
                    # B-slab: 64-offset partition-shifted copies of the block
                    # col range [klo, khi] in two SBUF->SBUF DMAs
                    xb = bpool.tile([P, ncol, GB, C], BF16, tag="xb", name=f"xb_{g}")
                    nc.sync.dma_start(out=xb[0:64], in_=xs[64:128, klo:khi + 1])
                    nc.sync.dma_start(out=xb[64:128], in_=xs[0:64, klo + 1:khi + 2])
                for ch in range(NCH):
                    if "m" in skip:
                        continue
                    ot = opool.tile([P, T], ODT, tag="ot", name=f"ot_{g}_{ch}")
                    ps = ppool.tile([P, T], F32, tag="ps", name=f"ps_{g}_{ch}")
                    for slab, col, woff, lo, hi, st, sp in instrs:
                        if slab == 0:
                            lhsT = xs[:, col, 2 * ch:2 * ch + 2, :]
                        else:
                            lhsT = xb[:, col - klo, 2 * ch:2 * ch + 2, :]
                        nc.tensor.matmul(
                            ps[:, lo:hi],
                            lhsT,
                            wt[:, woff:woff + (hi - lo)],
                            start=st,
                            stop=sp,
                        )
                    if ecut > 0:
                        nc.scalar.copy(out=ot[:, :ecut], in_=ps[:, :ecut])
                    if ecut < T:
                        nc.vector.tensor_scalar_mul(ot[:, ecut:], ps[:, ecut:], 1.0)
                    seng = getattr(nc, os.environ.get("KERNEL_V2_STQ", "gpsimd"))
                    seng.dma_start(out=out[g, ch], in_=ot[:])
    nc.compile()
    return nc


# ---------------------------------------------------------------------------
# host prep / dispatch
# ---------------------------------------------------------------------------

class PrepV1:
    def __init__(self, x, noise, scale, i0, frac, ca, sa, mask):
        import ml_dtypes

        wmat, sched = _build_w_blocks(i0, frac, scale)
        self.nblk = wmat.shape[0]
        self.sched = sched
        self.out_names = ["out"]
        dropped = [c for c in range(C) if not mask[c]]
        self.dropped = dropped
        self.rot_needed = bool(mask[0] or mask[1])
        noise_b = noise.astype(ml_dtypes.bfloat16)
        self.in_maps = []
        for core in range(N_CORES):
            b0 = core * BS
            rc = np.zeros((2, NG, P, P), np.float32)
            for g in range(NG):
                rc[0, g, :, :] = np.tile(ca[b0 + g * GB:b0 + (g + 1) * GB], NTB)[None, :]
                rc[1, g, :, :] = np.tile(sa[b0 + g * GB:b0 + (g + 1) * GB], NTB)[None, :]
            self.in_maps.append(
                {
                    "x": x[b0:b0 + BS],
                    "n": noise_b[b0:b0 + BS],
                    "w": wmat,
                    "rot": rc.reshape(2 * NG, P, P),
                }
            )

    def build(self, iters=1):
        return _build_nc_v1(self.nblk, self.sched, self.dropped, self.rot_needed, iters=iters)

    def postprocess_core0(self, out):
        o = out["out"].reshape(NG, NTB, P, GB, C)
        return np.ascontiguousarray(o.transpose(0, 3, 1, 2, 4)).reshape(BS, T, C).astype(np.float32)

    def postprocess(self, results):
        outs = []
        for r in results:
            o = r["out"].reshape(NG, NTB, P, GB, C)
            outs.append(np.ascontiguousarray(o.transpose(0, 3, 1, 2, 4)).reshape(BS, T, C))
        return np.concatenate(outs, axis=0).astype(np.float32, copy=False)


class PrepV2:
    def __init__(self, x, noise, scale, i0, frac, ca, sa, mask):
        import ml_dtypes

        wmat, instrs, zone_cols = _build_v2_schedule(i0, frac, scale)
        self.instrs = instrs
        self.zone_cols = zone_cols
        self.lw = wmat.shape[1]
        self.out_names = ["out"]
        self.rot_needed = bool(mask[0] or mask[1])
        # host-side channel dropout for non-rotation channels (and 0/1 too when
        # no rotation output survives); rotation channels are zeroed post-rot
        # on device if dropped
        hostdrop = [c for c in range(C) if not mask[c] and (c >= 2 or not self.rot_needed)]
        self.drop01 = [c for c in (0, 1) if not mask[c]] if self.rot_needed else []
        xb = x.astype(ml_dtypes.bfloat16)
        nb = (noise.astype(np.float32) * np.float32(JITTER_STD)).astype(ml_dtypes.bfloat16)
        if hostdrop:
            xb[:, :, hostdrop] = 0
            nb[:, :, hostdrop] = 0
        wmat_b = wmat.astype(ml_dtypes.bfloat16)
        self.in_maps = []
        for core in range(N_CORES):
            b0 = core * BS
            rc = np.zeros((2, NG, P, P), np.float32)
            for g in range(NG):
                rc[0, g, :, :] = np.tile(ca[b0 + g * GB:b0 + (g + 1) * GB], NTB)[None, :]
                rc[1, g, :, :] = np.tile(sa[b0 + g * GB:b0 + (g + 1) * GB], NTB)[None, :]
            self.in_maps.append(
                {
                    "x": xb[b0:b0 + BS],
                    "n": nb[b0:b0 + BS],
                    "w": wmat_b,
                    "rot": rc.reshape(2 * NG, P, P).astype(ml_dtypes.bfloat16),
                }
            )

    def build(self, iters=1):
        return _build_nc_v2(
            self.instrs, self.zone_cols, self.lw, self.rot_needed, self.drop01, iters=iters
        )

    def _one(self, o):
        # out [NG, NCH, P=(j2,c), T] -> (BS, T, C)
        o = np.asarray(o, dtype=np.float32).reshape(NG, NCH, 2, C, T)
        return np.ascontiguousarray(o.transpose(0, 1, 2, 4, 3)).reshape(BS, T, C)

    def postprocess_core0(self, out):
        return self._one(out["out"])

    def postprocess(self, results):
        return np.concatenate([self._one(r["out"]) for r in results], axis=0)


def prepare(x, noise, scale_u, warp_noise, angle_u, chmask_u):
    x = np.ascontiguousarray(np.asarray(x, dtype=np.float32))
    noise = np.ascontiguousarray(np.asarray(noise, dtype=np.float32))
    scale_u = np.asarray(scale_u, dtype=np.float32)
    warp_noise = np.asarray(warp_noise, dtype=np.float32)
    angle_u = np.asarray(angle_u, dtype=np.float32)
    chmask_u = np.asarray(chmask_u, dtype=np.float32)

    scale = np.float32(SCALE_LO) + scale_u[0] * np.float32(SCALE_HI - SCALE_LO)
    i0, frac = _warp_params(warp_noise)
    angle = (angle_u * np.float32(2.0 * np.pi) - np.float32(np.pi)).astype(np.float32)
    ca = np.cos(angle).astype(np.float32)
    sa = np.sin(angle).astype(np.float32)
    mask = np.asarray(chmask_u) > 0.1

    force = os.environ.get("KERNEL_FORCE", "")
    if force == "v1":
        return PrepV1(x, noise, scale, i0, frac, ca, sa, mask)
    try:
        return PrepV2(x, noise, scale, i0, frac, ca, sa, mask)
    except ValueError:
        if force == "v2":
            raise
        return PrepV1(x, noise, scale, i0, frac, ca, sa, mask)


def kernel(x, noise, scale_u, warp_noise, angle_u, chmask_u):
    prep = prepare(x, noise, scale_u, warp_noise, angle_u, chmask_u)
    iters = int(os.environ.get("KERNEL_ITERS", "1"))
    nc = prep.build(iters=iters)
    res = run_bass_kernel_spmd(nc, prep.in_maps, list(range(N_CORES)))
    return prep.postprocess([res.results[i] for i in range(N_CORES)])
